# revision 12
# baseline (speedup 1.0000x reference)
"""LSTM ActionEncoder kernel for Trainium2 (8 NeuronCores, data-parallel on batch).

Reference computation (T=20, B=2048, IN=512, H=1024):
    x_emb = obs @ W_emb.T + b_emb                      # [T, B, 512]
    per step: gates = x_t @ W_ih.T + h @ W_hh.T + b    # LSTM cell, i,f,g,o
    returns h_final                                    # [B, 1024]

Device algorithm (per core, B=256):
  * The embedding + input projection are two consecutive linear maps, so they
    are folded on the host into a single [3 -> 4096] map applied to
    (obs_x, obs_y, 1):  gates_x = obs_aug @ Wfold  (exact algebra, removes
    ~80% of the reference FLOPs).  The fold itself is O(weights) host prep.
  * Weight-moving matmul structure: the stationary operand is an hT block
    (reused across the whole 4096-wide gate dim -> one LDWEIGHTS per 4
    N=512 matmuls, so the PE runs at its streaming rate instead of being
    gated by per-matmul weight reloads), the moving operand is W_hh
    (pre-transposed/packed on the host, fp16).  gates land as
    [batch-chunk, gate-col] PSUM tiles, fp32 accumulation.
  * h/obs/W are fp16 (PE upconverts to FP22); c stays fp32 in SBUF.
  * The next step needs h transposed (hT is the stationary operand); h_new is
    fp16 so the 128x128 hardware XBAR DMA-transpose handles it off the PE.
  * Pointwise LSTM math runs on ACT (sigmoid/tanh) and DVE (products/sums),
    hidden under the PE's per-step matmul time.

Layouts (per core, b-chunk bc in {0,1}, gate g in {i,f,g,o}, v in {0,1} =
which 512-wide half of the 1024 h-columns, k = 128-row h chunk):
  wv_packed [16,128,2048]: [v*8+k, p, 512*g+c] = W_hh[1024*g+512*v+c, 128*k+p]
  wfold     [128,4096]: rows 0,1 = (W_ih@W_emb).T, row 2 = W_ih@b_emb+b_ih+b_hh
  obs_aug   [20,128,256]: rows 0,1 = obs_x, obs_y, row 2 = 1.0, rest 0
  h0t       [128,2048]: [p, 256*k+128*bc+w] = h0[128*bc+w, 128*k+p]
  c0b/h_out [2,128,1024]: [bc, p, hh] = state[128*bc+p, hh]
"""

import numpy as np

T = 20
H = 1024
NCORES = 8
B = 256  # batch per core

_CACHED_NC = None
LAST_RESULT = None  # BassKernelResults of the most recent run (for test harness)


def _build_program():
    import concourse.mybir as mybir
    import concourse.tile as tile
    from concourse import bacc

    f32 = mybir.dt.float32
    f16 = mybir.dt.float16
    AFT = mybir.ActivationFunctionType

    nc = bacc.Bacc("TRN2", target_bir_lowering=False)
    wv_dram = nc.dram_tensor("wv_packed", (16, 128, 2048), f16, kind="ExternalInput")
    wfold_dram = nc.dram_tensor("wfold", (128, 4096), f16, kind="ExternalInput")
    obs_dram = nc.dram_tensor("obs_aug", (T, 128, B), f16, kind="ExternalInput")
    h0_dram = nc.dram_tensor("h0t", (128, 2048), f16, kind="ExternalInput")
    c0_dram = nc.dram_tensor("c0b", (2, 128, 1024), f32, kind="ExternalInput")
    out_dram = nc.dram_tensor("h_out", (2, 128, 1024), f16, kind="ExternalOutput")

    with tile.TileContext(nc) as tc:
        with (
            tc.tile_pool(name="wpool", bufs=1) as wpool,
            tc.tile_pool(name="spool", bufs=1) as spool,
            tc.tile_pool(name="opool", bufs=2) as opool,
            tc.tile_pool(name="gpool", bufs=2) as gpool,
            tc.tile_pool(name="ppool", bufs=2, space="PSUM") as ppool,
        ):
            wfold_sb = wpool.tile([128, 4096], f16, name="wfold_sb")
            nc.sync.dma_start(wfold_sb[:], wfold_dram[:])

            # hT: stationary operand layout, ping-ponged across steps.
            # c: [b-chunk][128, 1024], updated in place.
            ht = [spool.tile([128, 2048], f16, name=f"ht{s}") for s in range(2)]
            cs = [spool.tile([128, 1024], f32, name=f"cs{bc}") for bc in range(2)]
            nc.scalar.dma_start(ht[0][:], h0_dram[:])
            for bc in range(2):
                nc.scalar.dma_start(cs[bc][:], c0_dram[bc])

            wv = {}
            for v in range(2):
                for k in range(8):
                    wtile = wpool.tile([128, 2048], f16, name=f"wv_{v}_{k}")
                    dma_eng = nc.sync if k % 2 == 0 else nc.scalar
                    dma_eng.dma_start(wtile[:], wv_dram[8 * v + k])
                    wv[(v, k)] = wtile

            for t in range(T):
                rd, wr = t % 2, (t + 1) % 2
                obs_t = opool.tile([128, B], f16, name="obs_t", tag="obs_t")
                nc.gpsimd.dma_start(obs_t[:], obs_dram[t])
                hnew = [
                    gpool.tile([128, 1024], f16, name=f"hnew{bc}", tag=f"hnew{bc}")
                    for bc in range(2)
                ]
                for v in range(2):
                    for bc in range(2):
                        ps = [
                            ppool.tile([128, 512], f32, name=f"ps{g}", tag=f"ps{g}")
                            for g in range(4)
                        ]
                        # Same-bank accumulation chains (g outer, k inner): the
                        # PE pays ~max(stream, LDW)+eps per MM within a chain,
                        # vs +46ns/MM when rotating PSUM banks every MM.
                        # The x-part (start=True) is h-independent, giving the
                        # PE ready work at the step boundary.
                        for g in range(4):
                            nc.tensor.matmul(
                                ps[g][:],
                                obs_t[32 * g : 32 * g + 32, 128 * bc : 128 * (bc + 1)],
                                wfold_sb[
                                    32 * g : 32 * g + 32,
                                    1024 * g + 512 * v : 1024 * g + 512 * v + 512,
                                ],
                                start=True,
                                stop=False,
                                tile_position=(32 * g, 0),
                            )
                        for g in range(4):
                            for k in range(8):
                                nc.tensor.matmul(
                                    ps[g][:],
                                    ht[rd][
                                        :, 256 * k + 128 * bc : 256 * k + 128 * bc + 128
                                    ],
                                    wv[(v, k)][:, 512 * g : 512 * (g + 1)],
                                    start=False,
                                    stop=(k == 7),
                                )
                        pi, pf, pg, po = ps
                        csl = cs[bc][:, 512 * v : 512 * (v + 1)]
                        # DVE may read at most ONE PSUM operand per op: route
                        # sigmoid(i), tanh(g) through SBUF; f, o stay in PSUM.
                        ab = gpool.tile([128, 1024], f32, name="ab", tag="ab")
                        a, b = ab[:, :512], ab[:, 512:]
                        nc.scalar.activation(a, pi[:], AFT.Sigmoid)
                        nc.scalar.activation(b, pg[:], AFT.Tanh)
                        nc.scalar.activation(pf[:], pf[:], AFT.Sigmoid)
                        nc.scalar.activation(po[:], po[:], AFT.Sigmoid)
                        nc.vector.tensor_mul(pf[:], pf[:], csl)  # f*c
                        nc.vector.tensor_mul(a, a, b)  # i*g
                        nc.vector.tensor_add(csl, pf[:], a)  # c_new
                        nc.scalar.activation(b, csl, AFT.Tanh)
                        nc.vector.tensor_mul(
                            hnew[bc][:, 512 * v : 512 * (v + 1)], po[:], b
                        )  # h_new (fp16)

                if t < T - 1:
                    # hT for the next step via 128x128 XBAR DMA transposes.
                    for bc in range(2):
                        for k in range(8):
                            nc.sync.dma_start(
                                ht[wr][
                                    :, 256 * k + 128 * bc : 256 * k + 128 * bc + 128
                                ],
                                hnew[bc][:, 128 * k : 128 * (k + 1)],
                                transpose=True,
                            )
                else:
                    for bc in range(2):
                        nc.sync.dma_start(out_dram[bc], hnew[bc][:])

    nc.compile()
    return nc


def _host_prep(inputs):
    obs = np.asarray(inputs["obs_traj"], dtype=np.float32)
    h0 = np.asarray(inputs["h0"], dtype=np.float32)
    c0 = np.asarray(inputs["c0"], dtype=np.float32)
    W_emb = np.asarray(inputs["W_emb"], dtype=np.float32)
    b_emb = np.asarray(inputs["b_emb"], dtype=np.float32)
    W_ih = np.asarray(inputs["W_ih"], dtype=np.float32)
    W_hh = np.asarray(inputs["W_hh"], dtype=np.float32)
    b_ih = np.asarray(inputs["b_ih"], dtype=np.float32)
    b_hh = np.asarray(inputs["b_hh"], dtype=np.float32)

    Wf = (W_ih @ W_emb).astype(np.float32)  # [4096, 2]
    biasf = (W_ih @ b_emb + b_ih + b_hh).astype(np.float32)
    wfold = np.zeros((128, 4096), np.float16)
    for r in range(4):
        wfold[32 * r + 0] = Wf[:, 0]
        wfold[32 * r + 1] = Wf[:, 1]
        wfold[32 * r + 2] = biasf

    # [v*8+k, p, 512*g+c] = W_hh[1024*g+512*v+c, 128*k+p]
    wv_packed = np.ascontiguousarray(
        W_hh.reshape(4, 2, 512, 8, 128).transpose(1, 3, 4, 0, 2).reshape(16, 128, 2048)
    ).astype(np.float16)

    in_maps = []
    for c in range(NCORES):
        sl = slice(B * c, B * (c + 1))
        obs_aug = np.zeros((T, 128, B), np.float16)
        for r in range(4):
            obs_aug[:, 32 * r + 0] = obs[:, sl, 0]
            obs_aug[:, 32 * r + 1] = obs[:, sl, 1]
            obs_aug[:, 32 * r + 2] = 1.0
        # h0t[p, 256k+128bc+w] = h0[128bc+w, 128k+p]
        h0t = np.ascontiguousarray(
            h0[sl].reshape(2, 128, 8, 128).transpose(3, 2, 0, 1).reshape(128, 2048)
        ).astype(np.float16)
        c0b = np.ascontiguousarray(c0[sl].reshape(2, 128, 1024))
        in_maps.append(
            dict(
                wv_packed=wv_packed,
                wfold=wfold,
                obs_aug=obs_aug,
                h0t=h0t,
                c0b=c0b,
            )
        )
    return in_maps


def _unpack_out(o):  # [2, 128, 1024] -> [256, 1024]
    return o.reshape(B, H)


def kernel(**inputs) -> np.ndarray:
    global _CACHED_NC, LAST_RESULT
    from concourse.bass_utils import run_bass_kernel_spmd

    in_maps = _host_prep(inputs)
    if _CACHED_NC is None:
        _CACHED_NC = _build_program()
    res = run_bass_kernel_spmd(_CACHED_NC, in_maps, core_ids=list(range(NCORES)))
    LAST_RESULT = res
    out = np.concatenate(
        [_unpack_out(res.results[c]["h_out"]) for c in range(NCORES)], axis=0
    )
    return np.ascontiguousarray(out.astype(np.float32))


# revision 13
# speedup vs baseline: 1.1800x; 1.1800x over previous
"""LSTM ActionEncoder kernel for Trainium2 (8 NeuronCores, data-parallel on batch).

Reference computation (T=20, B=2048, IN=512, H=1024):
    x_emb = obs @ W_emb.T + b_emb                      # [T, B, 512]
    per step: gates = x_t @ W_ih.T + h @ W_hh.T + b    # LSTM cell, i,f,g,o
    returns h_final                                    # [B, 1024]

Device algorithm (per core, B=256):
  * The embedding + input projection are two consecutive linear maps, so they
    are folded on the host into a single [3 -> 4096] map applied to
    (obs_x, obs_y, 1):  gates_x = obs_aug @ Wfold  (exact algebra, removes
    ~80% of the reference FLOPs).  The fold itself is O(weights) host prep.
  * Weight-moving matmul structure: the stationary operand is an hT block
    (reused across the whole 4096-wide gate dim -> one LDWEIGHTS per 4
    N=512 matmuls, so the PE runs at its streaming rate instead of being
    gated by per-matmul weight reloads), the moving operand is W_hh
    (pre-transposed/packed on the host, fp16).  gates land as
    [batch-chunk, gate-col] PSUM tiles, fp32 accumulation.
  * h/obs/W are fp16 (PE upconverts to FP22); c stays fp32 in SBUF.
  * The next step needs h transposed (hT is the stationary operand); h_new is
    fp16 so the 128x128 hardware XBAR DMA-transpose handles it off the PE.
  * Pointwise LSTM math runs on ACT (sigmoid/tanh) and DVE (products/sums),
    hidden under the PE's per-step matmul time.

Layouts (per core, b-chunk bc in {0,1}, gate g in {i,f,g,o}, v in {0,1} =
which 512-wide half of the 1024 h-columns, k = 128-row h chunk):
  wv_packed [16,128,2048]: [v*8+k, p, 512*g+c] = W_hh[1024*g+512*v+c, 128*k+p]
  wfold     [128,4096]: rows 0,1 = (W_ih@W_emb).T, row 2 = W_ih@b_emb+b_ih+b_hh
  obs_aug   [20,128,256]: rows 0,1 = obs_x, obs_y, row 2 = 1.0, rest 0
  h0t       [128,2048]: [p, 256*k+128*bc+w] = h0[128*bc+w, 128*k+p]
  c0b/h_out [2,128,1024]: [bc, p, hh] = state[128*bc+p, hh]
"""

import numpy as np

T = 20
H = 1024
NCORES = 8
B = 256  # batch per core

_CACHED_NC = None
LAST_RESULT = None  # BassKernelResults of the most recent run (for test harness)


def _build_program():
    import concourse.mybir as mybir
    import concourse.tile as tile
    from concourse import bacc

    f32 = mybir.dt.float32
    f16 = mybir.dt.float16
    AFT = mybir.ActivationFunctionType

    nc = bacc.Bacc("TRN2", target_bir_lowering=False)
    wv_dram = nc.dram_tensor("wv_packed", (16, 128, 2048), f16, kind="ExternalInput")
    wfold_dram = nc.dram_tensor("wfold", (128, 4096), f16, kind="ExternalInput")
    obs_dram = nc.dram_tensor("obs_aug", (T, 128, B), f16, kind="ExternalInput")
    h0_dram = nc.dram_tensor("h0t", (128, 2048), f16, kind="ExternalInput")
    c0_dram = nc.dram_tensor("c0b", (2, 128, 1024), f32, kind="ExternalInput")
    out_dram = nc.dram_tensor("h_out", (2, 128, 1024), f16, kind="ExternalOutput")

    with tile.TileContext(nc) as tc:
        with (
            tc.tile_pool(name="wpool", bufs=1) as wpool,
            tc.tile_pool(name="spool", bufs=1) as spool,
            tc.tile_pool(name="opool", bufs=2) as opool,
            tc.tile_pool(name="gpool", bufs=2) as gpool,
            tc.tile_pool(name="ppool", bufs=2, space="PSUM") as ppool,
        ):
            wfold_sb = wpool.tile([128, 4096], f16, name="wfold_sb")
            nc.sync.dma_start(wfold_sb[:], wfold_dram[:])

            # hT: stationary operand layout, ping-ponged across steps.
            # c: [b-chunk][128, 1024], updated in place.
            ht = [spool.tile([128, 2048], f16, name=f"ht{s}") for s in range(2)]
            cs = [spool.tile([128, 1024], f32, name=f"cs{bc}") for bc in range(2)]
            nc.scalar.dma_start(ht[0][:], h0_dram[:])

            wv = {}
            for v in range(2):
                for k in range(8):
                    wtile = wpool.tile([128, 2048], f16, name=f"wv_{v}_{k}")
                    dma_eng = nc.sync if k % 2 == 0 else nc.scalar
                    dma_eng.dma_start(wtile[:], wv_dram[8 * v + k])
                    wv[(v, k)] = wtile
                if v == 0:
                    # c is first consumed a few us into step 0; slot its load
                    # between the v=0 and v=1 weight batches.
                    for bc in range(2):
                        nc.scalar.dma_start(cs[bc][:], c0_dram[bc])

            for t in range(T):
                rd, wr = t % 2, (t + 1) % 2
                obs_t = opool.tile([128, B], f16, name="obs_t", tag="obs_t")
                nc.gpsimd.dma_start(obs_t[:], obs_dram[t])
                hnew = [
                    gpool.tile([128, 1024], f16, name=f"hnew{bc}", tag=f"hnew{bc}")
                    for bc in range(2)
                ]
                for bc in range(2):
                    for v in range(2):
                        ps = [
                            ppool.tile([128, 512], f32, name=f"ps{g}", tag=f"ps{g}")
                            for g in range(4)
                        ]
                        # Same-bank accumulation chains (g outer, k inner): the
                        # PE pays ~max(stream, LDW)+eps per MM within a chain,
                        # vs +46ns/MM when rotating PSUM banks every MM.
                        # The x-part (start=True) is h-independent, giving the
                        # PE ready work at the step boundary.
                        for g in range(4):
                            nc.tensor.matmul(
                                ps[g][:],
                                obs_t[32 * g : 32 * g + 32, 128 * bc : 128 * (bc + 1)],
                                wfold_sb[
                                    32 * g : 32 * g + 32,
                                    1024 * g + 512 * v : 1024 * g + 512 * v + 512,
                                ],
                                start=True,
                                stop=False,
                                tile_position=(32 * g, 0),
                            )
                        for g in range(4):
                            for k in range(8):
                                nc.tensor.matmul(
                                    ps[g][:],
                                    ht[rd][
                                        :, 256 * k + 128 * bc : 256 * k + 128 * bc + 128
                                    ],
                                    wv[(v, k)][:, 512 * g : 512 * (g + 1)],
                                    start=False,
                                    stop=(k == 7),
                                )
                        pi, pf, pg, po = ps
                        csl = cs[bc][:, 512 * v : 512 * (v + 1)]
                        # DVE may read at most ONE PSUM operand per op: route
                        # sigmoid(i), tanh(g) through SBUF; f, o stay in PSUM.
                        ab = gpool.tile([128, 1024], f32, name="ab", tag="ab")
                        a, b = ab[:, :512], ab[:, 512:]
                        nc.scalar.activation(a, pi[:], AFT.Sigmoid)
                        nc.scalar.activation(b, pg[:], AFT.Tanh)
                        nc.scalar.activation(pf[:], pf[:], AFT.Sigmoid)
                        nc.scalar.activation(po[:], po[:], AFT.Sigmoid)
                        nc.vector.tensor_mul(pf[:], pf[:], csl)  # f*c
                        nc.vector.tensor_mul(a, a, b)  # i*g
                        nc.vector.tensor_add(csl, pf[:], a)  # c_new
                        nc.scalar.activation(b, csl, AFT.Tanh)
                        nc.vector.tensor_mul(
                            hnew[bc][:, 512 * v : 512 * (v + 1)], po[:], b
                        )  # h_new (fp16)
                        if t < T - 1:
                            # hT blocks for the next step via 128x128 XBAR DMA
                            # transposes, emitted as soon as this quadrant's
                            # h_new half exists.
                            for k in range(4 * v, 4 * v + 4):
                                nc.sync.dma_start(
                                    ht[wr][
                                        :, 256 * k + 128 * bc : 256 * k + 128 * bc + 128
                                    ],
                                    hnew[bc][:, 128 * k : 128 * (k + 1)],
                                    transpose=True,
                                )

                if t == T - 1:
                    for bc in range(2):
                        nc.sync.dma_start(out_dram[bc], hnew[bc][:])

    nc.compile()
    return nc


def _host_prep(inputs):
    obs = np.asarray(inputs["obs_traj"], dtype=np.float32)
    h0 = np.asarray(inputs["h0"], dtype=np.float32)
    c0 = np.asarray(inputs["c0"], dtype=np.float32)
    W_emb = np.asarray(inputs["W_emb"], dtype=np.float32)
    b_emb = np.asarray(inputs["b_emb"], dtype=np.float32)
    W_ih = np.asarray(inputs["W_ih"], dtype=np.float32)
    W_hh = np.asarray(inputs["W_hh"], dtype=np.float32)
    b_ih = np.asarray(inputs["b_ih"], dtype=np.float32)
    b_hh = np.asarray(inputs["b_hh"], dtype=np.float32)

    Wf = (W_ih @ W_emb).astype(np.float32)  # [4096, 2]
    biasf = (W_ih @ b_emb + b_ih + b_hh).astype(np.float32)
    wfold = np.zeros((128, 4096), np.float16)
    for r in range(4):
        wfold[32 * r + 0] = Wf[:, 0]
        wfold[32 * r + 1] = Wf[:, 1]
        wfold[32 * r + 2] = biasf

    # [v*8+k, p, 512*g+c] = W_hh[1024*g+512*v+c, 128*k+p]
    wv_packed = np.ascontiguousarray(
        W_hh.reshape(4, 2, 512, 8, 128).transpose(1, 3, 4, 0, 2).reshape(16, 128, 2048)
    ).astype(np.float16)

    in_maps = []
    for c in range(NCORES):
        sl = slice(B * c, B * (c + 1))
        obs_aug = np.zeros((T, 128, B), np.float16)
        for r in range(4):
            obs_aug[:, 32 * r + 0] = obs[:, sl, 0]
            obs_aug[:, 32 * r + 1] = obs[:, sl, 1]
            obs_aug[:, 32 * r + 2] = 1.0
        # h0t[p, 256k+128bc+w] = h0[128bc+w, 128k+p]
        h0t = np.ascontiguousarray(
            h0[sl].reshape(2, 128, 8, 128).transpose(3, 2, 0, 1).reshape(128, 2048)
        ).astype(np.float16)
        c0b = np.ascontiguousarray(c0[sl].reshape(2, 128, 1024))
        in_maps.append(
            dict(
                wv_packed=wv_packed,
                wfold=wfold,
                obs_aug=obs_aug,
                h0t=h0t,
                c0b=c0b,
            )
        )
    return in_maps


def _unpack_out(o):  # [2, 128, 1024] -> [256, 1024]
    return o.reshape(B, H)


def kernel(**inputs) -> np.ndarray:
    global _CACHED_NC, LAST_RESULT
    from concourse.bass_utils import run_bass_kernel_spmd

    in_maps = _host_prep(inputs)
    if _CACHED_NC is None:
        _CACHED_NC = _build_program()
    res = run_bass_kernel_spmd(_CACHED_NC, in_maps, core_ids=list(range(NCORES)))
    LAST_RESULT = res
    out = np.concatenate(
        [_unpack_out(res.results[c]["h_out"]) for c in range(NCORES)], axis=0
    )
    return np.ascontiguousarray(out.astype(np.float32))


# revision 14
# speedup vs baseline: 1.1858x; 1.0049x over previous
"""LSTM ActionEncoder kernel for Trainium2 (8 NeuronCores, data-parallel on batch).

Reference computation (T=20, B=2048, IN=512, H=1024):
    x_emb = obs @ W_emb.T + b_emb                      # [T, B, 512]
    per step: gates = x_t @ W_ih.T + h @ W_hh.T + b    # LSTM cell, i,f,g,o
    returns h_final                                    # [B, 1024]

Device algorithm (per core, B=256):
  * The embedding + input projection are two consecutive linear maps, so they
    are folded on the host into a single [3 -> 4096] map applied to
    (obs_x, obs_y, 1):  gates_x = obs_aug @ Wfold  (exact algebra, removes
    ~80% of the reference FLOPs).  The fold itself is O(weights) host prep.
  * Weight-moving matmul structure: the stationary operand is an hT block
    (reused across the whole 4096-wide gate dim -> one LDWEIGHTS per 4
    N=512 matmuls, so the PE runs at its streaming rate instead of being
    gated by per-matmul weight reloads), the moving operand is W_hh
    (pre-transposed/packed on the host, fp16).  gates land as
    [batch-chunk, gate-col] PSUM tiles, fp32 accumulation.
  * h/obs/W are fp16 (PE upconverts to FP22); c stays fp32 in SBUF.
  * The next step needs h transposed (hT is the stationary operand); h_new is
    fp16 so the 128x128 hardware XBAR DMA-transpose handles it off the PE.
  * Pointwise LSTM math runs on ACT (sigmoid/tanh) and DVE (products/sums),
    hidden under the PE's per-step matmul time.

Layouts (per core, b-chunk bc in {0,1}, gate g in {i,f,g,o}, v in {0,1} =
which 512-wide half of the 1024 h-columns, k = 128-row h chunk):
  wv_packed [16,128,2048]: [v*8+k, p, 512*g+c] = W_hh[1024*g+512*v+c, 128*k+p]
  wfold     [128,4096]: rows 0,1 = (W_ih@W_emb).T, row 2 = W_ih@b_emb+b_ih+b_hh
  obs_aug   [20,128,256]: rows 0,1 = obs_x, obs_y, row 2 = 1.0, rest 0
  h0t       [128,2048]: [p, 256*k+128*bc+w] = h0[128*bc+w, 128*k+p]
  c0b/h_out [2,128,1024]: [bc, p, hh] = state[128*bc+p, hh]
"""

import numpy as np

T = 20
H = 1024
NCORES = 8
B = 256  # batch per core

_CACHED_NC = None
LAST_RESULT = None  # BassKernelResults of the most recent run (for test harness)


def _build_program():
    import concourse.mybir as mybir
    import concourse.tile as tile
    from concourse import bacc

    f32 = mybir.dt.float32
    f16 = mybir.dt.float16
    AFT = mybir.ActivationFunctionType

    nc = bacc.Bacc("TRN2", target_bir_lowering=False)
    wv_dram = nc.dram_tensor("wv_packed", (16, 128, 2048), f16, kind="ExternalInput")
    wfold_dram = nc.dram_tensor("wfold", (128, 4096), f16, kind="ExternalInput")
    obs_dram = nc.dram_tensor("obs_aug", (T, 128, B), f16, kind="ExternalInput")
    h0_dram = nc.dram_tensor("h0t", (128, 2048), f16, kind="ExternalInput")
    c0_dram = nc.dram_tensor("c0b", (2, 128, 1024), f32, kind="ExternalInput")
    out_dram = nc.dram_tensor("h_out", (2, 128, 1024), f16, kind="ExternalOutput")

    with tile.TileContext(nc) as tc:
        with (
            tc.tile_pool(name="wpool", bufs=1) as wpool,
            tc.tile_pool(name="spool", bufs=1) as spool,
            tc.tile_pool(name="opool", bufs=2) as opool,
            tc.tile_pool(name="gpool", bufs=2) as gpool,
            tc.tile_pool(name="ppool", bufs=2, space="PSUM") as ppool,
        ):
            wfold_sb = wpool.tile([128, 4096], f16, name="wfold_sb")
            nc.sync.dma_start(wfold_sb[:], wfold_dram[:])

            # hT: stationary operand layout, ping-ponged across steps.
            # c: [b-chunk][128, 1024], updated in place.
            ht = [spool.tile([128, 2048], f16, name=f"ht{s}") for s in range(2)]
            cs = [spool.tile([128, 1024], f32, name=f"cs{bc}") for bc in range(2)]
            nc.scalar.dma_start(ht[0][:], h0_dram[:])

            wv = {}
            for v in range(2):
                for k in range(8):
                    wtile = wpool.tile([128, 2048], f16, name=f"wv_{v}_{k}")
                    dma_eng = nc.sync if k % 2 == 0 else nc.scalar
                    dma_eng.dma_start(wtile[:], wv_dram[8 * v + k])
                    wv[(v, k)] = wtile
                if v == 0:
                    # c is first consumed a few us into step 0; slot its load
                    # between the v=0 and v=1 weight batches.
                    for bc in range(2):
                        nc.scalar.dma_start(cs[bc][:], c0_dram[bc])

            # PE warm-up: ~24 dummy matmuls on the first-arriving tile
            # (wfold) bridge the initial weight-DMA wait and get the HAM
            # clock gate to 8/8 before real work lands. Output is garbage
            # in a cycled PSUM slot that is never read.
            warm = ppool.tile([128, 512], f32, name="ps0w", tag="ps0")
            for i in range(24):
                nc.tensor.matmul(
                    warm[:],
                    wfold_sb[:, 128 * i : 128 * (i + 1)],
                    wfold_sb[:, 512 * (i % 8) : 512 * (i % 8 + 1)],
                    start=True,
                    stop=True,
                )

            for t in range(T):
                rd, wr = t % 2, (t + 1) % 2
                obs_t = opool.tile([128, B], f16, name="obs_t", tag="obs_t")
                nc.gpsimd.dma_start(obs_t[:], obs_dram[t])
                hnew = [
                    gpool.tile([128, 1024], f16, name=f"hnew{bc}", tag=f"hnew{bc}")
                    for bc in range(2)
                ]
                for bc in range(2):
                    for v in range(2):
                        ps = [
                            ppool.tile([128, 512], f32, name=f"ps{g}", tag=f"ps{g}")
                            for g in range(4)
                        ]
                        # Same-bank accumulation chains (g outer, k inner): the
                        # PE pays ~max(stream, LDW)+eps per MM within a chain,
                        # vs +46ns/MM when rotating PSUM banks every MM.
                        # The x-part (start=True) is h-independent, giving the
                        # PE ready work at the step boundary.
                        for g in range(4):
                            nc.tensor.matmul(
                                ps[g][:],
                                obs_t[32 * g : 32 * g + 32, 128 * bc : 128 * (bc + 1)],
                                wfold_sb[
                                    32 * g : 32 * g + 32,
                                    1024 * g + 512 * v : 1024 * g + 512 * v + 512,
                                ],
                                start=True,
                                stop=False,
                                tile_position=(32 * g, 0),
                            )
                        for g in range(4):
                            for k in range(8):
                                nc.tensor.matmul(
                                    ps[g][:],
                                    ht[rd][
                                        :, 256 * k + 128 * bc : 256 * k + 128 * bc + 128
                                    ],
                                    wv[(v, k)][:, 512 * g : 512 * (g + 1)],
                                    start=False,
                                    stop=(k == 7),
                                )
                        pi, pf, pg, po = ps
                        csl = cs[bc][:, 512 * v : 512 * (v + 1)]
                        # DVE may read at most ONE PSUM operand per op: route
                        # sigmoid(i), tanh(g) through SBUF; f, o stay in PSUM.
                        ab = gpool.tile([128, 1024], f32, name="ab", tag="ab")
                        a, b = ab[:, :512], ab[:, 512:]
                        nc.scalar.activation(a, pi[:], AFT.Sigmoid)
                        nc.scalar.activation(b, pg[:], AFT.Tanh)
                        nc.scalar.activation(pf[:], pf[:], AFT.Sigmoid)
                        nc.scalar.activation(po[:], po[:], AFT.Sigmoid)
                        nc.vector.tensor_mul(pf[:], pf[:], csl)  # f*c
                        nc.vector.tensor_mul(a, a, b)  # i*g
                        nc.vector.tensor_add(csl, pf[:], a)  # c_new
                        nc.scalar.activation(b, csl, AFT.Tanh)
                        nc.vector.tensor_mul(
                            hnew[bc][:, 512 * v : 512 * (v + 1)], po[:], b
                        )  # h_new (fp16)
                        if t < T - 1:
                            # hT blocks for the next step via 128x128 XBAR DMA
                            # transposes, emitted as soon as this quadrant's
                            # h_new half exists.
                            for k in range(4 * v, 4 * v + 4):
                                nc.sync.dma_start(
                                    ht[wr][
                                        :, 256 * k + 128 * bc : 256 * k + 128 * bc + 128
                                    ],
                                    hnew[bc][:, 128 * k : 128 * (k + 1)],
                                    transpose=True,
                                )

                if t == T - 1:
                    for bc in range(2):
                        nc.sync.dma_start(out_dram[bc], hnew[bc][:])

    nc.compile()
    return nc


def _host_prep(inputs):
    obs = np.asarray(inputs["obs_traj"], dtype=np.float32)
    h0 = np.asarray(inputs["h0"], dtype=np.float32)
    c0 = np.asarray(inputs["c0"], dtype=np.float32)
    W_emb = np.asarray(inputs["W_emb"], dtype=np.float32)
    b_emb = np.asarray(inputs["b_emb"], dtype=np.float32)
    W_ih = np.asarray(inputs["W_ih"], dtype=np.float32)
    W_hh = np.asarray(inputs["W_hh"], dtype=np.float32)
    b_ih = np.asarray(inputs["b_ih"], dtype=np.float32)
    b_hh = np.asarray(inputs["b_hh"], dtype=np.float32)

    Wf = (W_ih @ W_emb).astype(np.float32)  # [4096, 2]
    biasf = (W_ih @ b_emb + b_ih + b_hh).astype(np.float32)
    wfold = np.zeros((128, 4096), np.float16)
    for r in range(4):
        wfold[32 * r + 0] = Wf[:, 0]
        wfold[32 * r + 1] = Wf[:, 1]
        wfold[32 * r + 2] = biasf

    # [v*8+k, p, 512*g+c] = W_hh[1024*g+512*v+c, 128*k+p]
    wv_packed = np.ascontiguousarray(
        W_hh.reshape(4, 2, 512, 8, 128).transpose(1, 3, 4, 0, 2).reshape(16, 128, 2048)
    ).astype(np.float16)

    in_maps = []
    for c in range(NCORES):
        sl = slice(B * c, B * (c + 1))
        obs_aug = np.zeros((T, 128, B), np.float16)
        for r in range(4):
            obs_aug[:, 32 * r + 0] = obs[:, sl, 0]
            obs_aug[:, 32 * r + 1] = obs[:, sl, 1]
            obs_aug[:, 32 * r + 2] = 1.0
        # h0t[p, 256k+128bc+w] = h0[128bc+w, 128k+p]
        h0t = np.ascontiguousarray(
            h0[sl].reshape(2, 128, 8, 128).transpose(3, 2, 0, 1).reshape(128, 2048)
        ).astype(np.float16)
        c0b = np.ascontiguousarray(c0[sl].reshape(2, 128, 1024))
        in_maps.append(
            dict(
                wv_packed=wv_packed,
                wfold=wfold,
                obs_aug=obs_aug,
                h0t=h0t,
                c0b=c0b,
            )
        )
    return in_maps


def _unpack_out(o):  # [2, 128, 1024] -> [256, 1024]
    return o.reshape(B, H)


def kernel(**inputs) -> np.ndarray:
    global _CACHED_NC, LAST_RESULT
    from concourse.bass_utils import run_bass_kernel_spmd

    in_maps = _host_prep(inputs)
    if _CACHED_NC is None:
        _CACHED_NC = _build_program()
    res = run_bass_kernel_spmd(_CACHED_NC, in_maps, core_ids=list(range(NCORES)))
    LAST_RESULT = res
    out = np.concatenate(
        [_unpack_out(res.results[c]["h_out"]) for c in range(NCORES)], axis=0
    )
    return np.ascontiguousarray(out.astype(np.float32))
